# revision 11
# baseline (speedup 1.0000x reference)
"""Category-specific linear (MoE-style routed batched matmul) on 8 trn2 cores.

out[b, s, h] = sum_i x[b, s, i] * W[cat_ids[b], i, h] + bias[cat_ids[b], h]

Shapes (hardcoded): x (32, 512, 1024) f32, cat_ids (32,) int, W (16, 1024, 4096)
f32, b (16, 4096) f32 -> out (32, 512, 4096) f32.

Strategy: data-parallel over batch, 4 batches per core, with host-side routing
that always packs one same-category PAIR of batches plus two singles per core
(slot capacities [2, 1, 1] batches). With 32 batches over 16 categories there
are always >= (32 - 16)/2 = 8 disjoint same-category pairs, so this packing is
feasible for ANY cat_ids. Each core then loads only 3 weight matrices (24 MB
fp16) instead of 4, keeping the kernel compute-bound.

Per core, over sections (slot, quarter) in slot-major order:
  stream W[slot]-quarter as 8 tiles [128, 1024] (256 KB DMAs, sync ring),
  prefetched one full section ahead;
  for m over the slot's 128-sample tiles (8 for slot 0, 4 for 1/2):
    for kt(8) x n4(2): fp16 matmul -> psum[n4] (accumulate over kt)
    evict: DVE adds bias into a [128, 1024] f16 tile, one 256 KB DMA to out
    (f16; host upcasts to f32)

Schedule notes (trace-derived):
- The fp16 PE floor is 1024 matmuls x ~216 ns = 221.5 us. fp8 DoubleRow was
  measured on hw at the SAME per-matmul interval as fp16 (2x flops via paired
  K), so no fp8 mixture beats fp16 under the 2e-2 error gate (e4m3 data
  streams alone cost 4.3e-2 and corrections eat the entire 2x).
- Each dma_start costs ~0.6-1.4 us of ISSUING-ENGINE time, so tile size /
  DMA count matters as much as bytes: W stays at 8x256KB per section, out is
  one 256 KB DMA per m-tile-section (not two 128 KB).
- Cold start: the first matmul needs only W0 + xt chunk0; xt c0 is emitted
  right after W0 so both race on separate rings (~1.6 us) instead of c0
  queueing behind all of section 0 in the HWDGE lane rotation (first real
  matmul ~8.5 us vs 11.5 us). The quad (kt-outer over 4 m-tiles) then
  consumes each W tile with 8 matmuls (1.73 us) against ~1.5 us supply.
- Warmup is 3 narrow (256-col) fp16 matmuls: enough PE activity to start the
  pstate ramp + HAM un-throttle window, short enough to finish before the
  first W/xt tiles land. A PE idle gap resets the pstate ramp (observed:
  ~430 ns mid-pstate issue intervals for ~3 us after any >2 us stall), so
  the cold path is arranged to keep the PE continuously fed once started.
- Tail: the last m-tile runs n4-outer so chunk 0's psum group closes 8
  matmuls early; its evict+DMA (scalar ring) overlap the final matmuls and
  the tail carries only chunk 1's chain on the otherwise-idle sync ring.
- fp16 everywhere keeps FWL enabled (LDWEIGHTS hides behind the previous
  matmul); steady-state issue interval ~216 ns. Relative error ~3e-4 from
  fp16 inputs + ~5e-4 from the f16 output rounding, far under the 2e-2 gate.
"""

import numpy as np

import concourse.bacc as bacc
import concourse.mybir as mybir
import concourse.bass as bass
import concourse.tile as tile
from concourse.bass_utils import run_bass_kernel_spmd

N_CORES = 8
B, S, K, H = 32, 512, 1024, 4096
BPC = B // N_CORES          # batches per core
P = 128                     # partitions
KT = K // P                 # k tiles (8)
MT = S // P                 # sample tiles per batch (4)
NSEC = 4                    # n quarter-sections per slot
NH = H // NSEC              # cols per quarter (1024)
NMM = NH // 512             # 512-wide matmuls per quarter (2)
XC = 4                      # xt chunks per batch
XKT = KT // XC              # k tiles per xt chunk (2)
SLOT_BATCHES = (2, 1, 1)    # batches per weight slot
NSLOT = len(SLOT_BATCHES)
N_WARM = 3                  # fp16 warmup matmuls (pstate ramp + HAM window)
WARM_N = 256                # warmup matmul width (ends before tiles land)

_COMPILED = None


def _build():
    nc = bacc.Bacc("TRN2", target_bir_lowering=False, debug=False)
    f32 = mybir.dt.float32
    f16 = mybir.dt.float16

    xt_ap = nc.dram_tensor("xt", [BPC, K, S], f16, kind="ExternalInput").ap()
    w_ap = nc.dram_tensor("w", [NSLOT, K, H], f16, kind="ExternalInput").ap()
    bias_ap = nc.dram_tensor("bias", [NSLOT, H], f32, kind="ExternalInput").ap()
    out_ap = nc.dram_tensor("out", [BPC, S, H], f16, kind="ExternalOutput").ap()

    sections = [(s, q) for s in range(NSLOT) for q in range(NSEC)]
    slot_base = [sum(SLOT_BATCHES[:s]) for s in range(NSLOT)]

    with tile.TileContext(nc) as tc:
        with (
            tc.tile_pool(name="xt_pool", bufs=BPC * XC + 1) as xt_pool,
            tc.tile_pool(name="w_pool", bufs=24) as w_pool,
            tc.tile_pool(name="bias_pool", bufs=2) as bias_pool,
            tc.tile_pool(name="out_pool", bufs=8) as out_pool,
            tc.tile_pool(name="ps_pool", bufs=8, space="PSUM") as ps_pool,
        ):
            # Short fp16 warmup: starts the PE pstate ramp + HAM un-throttle
            # window while the first DMAs land. Result read once so DCE
            # keeps it.
            warm_x = xt_pool.tile([P, P], f16, name="warm_x", tag="warm")
            warm_w = w_pool.tile([P, WARM_N], f16, tag="w", name="warm_w")
            nc.vector.memset(warm_x[:], 0.0)
            nc.vector.memset(warm_w[:], 0.0)
            warm_ps = ps_pool.tile([P, WARM_N], f32, tag="ps", name="warm_ps")
            for _ in range(N_WARM):
                nc.tensor.matmul(
                    warm_ps[:], warm_x[:], warm_w[:], start=True, stop=True,
                    skip_group_check=True,
                )
            warm_out = out_pool.tile([P, 4], f16, name="warm_out", tag="warmo")
            nc.vector.tensor_copy(warm_out[:], warm_ps[:, 0:4])

            def evict(ps, bias_t, b, mm, half):
                """psum -> (+bias on DVE) -> f16 sbuf -> one 256 KB out DMA."""
                out_t = out_pool.tile([P, NH], f16)
                for n4 in range(NMM):
                    nc.vector.tensor_add(
                        out_t[:, n4 * 512 : (n4 + 1) * 512], ps[n4][:],
                        bias_t[:, n4 * 512 : (n4 + 1) * 512],
                    )
                nc.scalar.dma_start(
                    out_ap[
                        b,
                        mm * P : (mm + 1) * P,
                        half * NH : (half + 1) * NH,
                    ],
                    out_t[:],
                )

            def evict_chunk(ps_t, bias_t, b, mm, half, n4, ring):
                """Single 512-col chunk evict for the tail."""
                out_t = out_pool.tile([P, NH], f16)
                nc.vector.tensor_add(
                    out_t[:, 0:512], ps_t[:],
                    bias_t[:, n4 * 512 : (n4 + 1) * 512],
                )
                ring.dma_start(
                    out_ap[
                        b,
                        mm * P : (mm + 1) * P,
                        half * NH + n4 * 512 : half * NH + (n4 + 1) * 512,
                    ],
                    out_t[:, 0:512],
                )

            def fetch_w(sec, kt, dep=None):
                s, half = sec
                w_r = w_ap[s].rearrange("(kt p) n -> p kt n", p=P)
                w_t = w_pool.tile([P, NH], f16, tag="w", name="w_t")
                if dep is not None:
                    # RAW-on-dep + WAW-on-this-tile marker: this DMA can
                    # only launch after dep's data has landed. Cold-window
                    # transfers otherwise all race at packet-level
                    # round-robin and ALL finish late together.
                    nc.vector.tensor_copy(w_t[:, 0:8], dep[:, 0:8])
                nc.sync.dma_start(w_t[:], w_r[:, kt, half * NH : (half + 1) * NH])
                return w_t

            def fetch_bias(sec, gate=False):
                s, half = sec
                bias_t = bias_pool.tile([P, NH], f32, name="bias_t")
                if gate:
                    # WAW marker: the DMA must wait for this DVE memset,
                    # which (by DVE program order) runs only after the cold
                    # window — keeps the scheduler from hoisting the
                    # transfer into the bandwidth-limited start.
                    nc.vector.memset(bias_t[:, 0:8], 0.0)
                bias_src = bias_ap[s, half * NH : (half + 1) * NH]
                nc.gpsimd.dma_start(
                    out=bias_t[:],
                    in_=bass.AP(
                        tensor=bias_src.tensor,
                        offset=bias_src.offset,
                        ap=[[0, P]] + list(bias_src.ap),
                    ),
                )
                return bias_t

            xt_ts = {}  # global batch index -> [XC chunk tiles]

            def xt_chunk(gb, c, gate=False, dep=None):
                xt_t = xt_pool.tile([P, XKT, S], f16, name="xt_t", tag="xt")
                if gate:
                    nc.vector.memset(xt_t[:, 0, 0:8], 0.0)
                if dep is not None:
                    nc.vector.tensor_copy(xt_t[:, 0, 0:8], dep[:, 0, 0:8])
                nc.scalar.dma_start(
                    xt_t[:],
                    xt_ap[gb, c * XKT * P : (c + 1) * XKT * P, :].rearrange(
                        "(kt p) m -> p kt m", p=P
                    ),
                )
                return xt_t

            def ensure_xt(gb, gate=False):
                if gb in xt_ts:
                    return
                xt_ts[gb] = [xt_chunk(gb, c, gate) for c in range(XC)]

            def lhsT_of(gb, kt, mm):
                c, ktl = divmod(kt, XKT)
                return xt_ts[gb][c][:, ktl, mm * P : (mm + 1) * P]

            # Cold window: W0 (sync ring) and batch-0 xt chunk0 (scalar
            # ring) launch alone and split the full HBM bandwidth — the
            # first matmul unblocks at ~8.4 us instead of ~12.5. W1 and c1
            # carry dep markers on W0/c0, and since each engine issues its
            # queue in order, that single gate holds ALL later cold
            # transfers off the wire until stage 0 has landed; afterwards
            # they stream at engine-issue rate (~0.7 us apart), each
            # arriving ahead of the quad's ~1.73 us/tile consumption.
            with tc.high_priority():
                w0 = fetch_w(sections[0], 0)
                c0 = xt_chunk(0, 0)
                cur_w = [w0, fetch_w(sections[0], 1, dep=w0)]
                cur_w += [fetch_w(sections[0], kt) for kt in range(2, KT)]
                xt_ts[0] = [c0, xt_chunk(0, 1, dep=c0)] + [
                    xt_chunk(0, c) for c in range(2, XC)
                ]
                cur_bias = fetch_bias(sections[0])

            for si, (s, half) in enumerate(sections):
                nb = SLOT_BATCHES[s]
                bi0 = slot_base[s]
                w_tiles, bias_t = cur_w, cur_bias
                if 0 < si < len(sections) - 1:
                    cur_w = [fetch_w(sections[si + 1], kt) for kt in range(KT)]
                    cur_bias = fetch_bias(sections[si + 1])

                m0 = 0
                if si == 0:
                    # Cold start: kt-outer over a QUAD of m-tiles (4 m x 2
                    # psum banks) so each 256 KB W tile feeds 8 matmuls
                    # (~1.73 us) vs its ~1.5 us shared-bandwidth DMA.
                    ps2 = [
                        [
                            ps_pool.tile([P, 512], f32, tag="ps", name="ps")
                            for _ in range(NMM)
                        ]
                        for _ in range(4)
                    ]
                    for kt in range(KT):
                        for mi in range(4):
                            lhsT = lhsT_of(bi0, kt, mi)
                            for n4 in range(NMM):
                                nc.tensor.matmul(
                                    ps2[mi][n4][:],
                                    lhsT,
                                    w_tiles[kt][:, n4 * 512 : (n4 + 1) * 512],
                                    start=(kt == 0),
                                    stop=(kt == KT - 1),
                                )
                            if kt == KT - 1:
                                # evict mi's banks right away so the next
                                # m-tile's matmuls get psum banks sooner
                                evict(ps2[mi], bias_t, bi0, mi, half)
                    m0 = 4
                    # Batch 1's xt streams during the quad (the quad's 2x
                    # demand margin tolerates the bandwidth sharing) so it is
                    # ready when m=4 starts right after.
                    ensure_xt(bi0 + 1)
                    # Section 1's prefetch is emitted only now, with a gated
                    # bias, so none of it competes with the cold window.
                    cur_w = [fetch_w(sections[1], kt) for kt in range(KT)]
                    cur_bias = fetch_bias(sections[1], gate=True)

                for m in range(m0, nb * MT):
                    b, mm = divmod(m, MT)
                    if half == 0 and b + 1 < nb and m == m0:
                        # start the next batch's xt stream with lead time
                        ensure_xt(bi0 + b + 1, gate=True)
                    if half == 0 and s + 1 < NSLOT and m == m0 + 1:
                        # and the next slot's first batch
                        ensure_xt(slot_base[s + 1], gate=True)
                    ps = [
                        ps_pool.tile([P, 512], f32, tag="ps", name="ps")
                        for _ in range(NMM)
                    ]
                    last = si == len(sections) - 1 and m == nb * MT - 1
                    if last:
                        # Final m-tile: n4-outer so chunk 0's psum group
                        # closes 8 matmuls early — its evict + out DMA
                        # (scalar ring) overlap the remaining matmuls, and
                        # the tail carries only chunk 1's chain (sync ring).
                        for n4 in range(NMM):
                            for kt in range(KT):
                                nc.tensor.matmul(
                                    ps[n4][:],
                                    lhsT_of(bi0 + b, kt, mm),
                                    w_tiles[kt][:, n4 * 512 : (n4 + 1) * 512],
                                    start=(kt == 0),
                                    stop=(kt == KT - 1),
                                )
                            evict_chunk(
                                ps[n4], bias_t, bi0 + b, mm, half, n4,
                                ring=nc.sync if n4 == NMM - 1 else nc.scalar,
                            )
                    else:
                        for kt in range(KT):
                            lhsT = lhsT_of(bi0 + b, kt, mm)
                            for n4 in range(NMM):
                                nc.tensor.matmul(
                                    ps[n4][:],
                                    lhsT,
                                    w_tiles[kt][:, n4 * 512 : (n4 + 1) * 512],
                                    start=(kt == 0),
                                    stop=(kt == KT - 1),
                                )
                        evict(ps, bias_t, bi0 + b, mm, half)
    nc.compile()
    return nc


def _get_compiled():
    global _COMPILED
    if _COMPILED is None:
        _COMPILED = _build()
    return _COMPILED


def _pack(cat_ids):
    """Assign batches to cores with slot capacities [2,1,1] per core.

    Returns per-core (idx, slot_cats): idx = 4 batch indices ordered
    [pair0, pair1, single_b, single_c]; slot_cats = categories for the 3 slots.
    Always feasible: #disjoint same-cat pairs = (32 - #odd-count cats)/2 >= 8.
    """
    cat_ids = np.asarray(cat_ids)
    by_cat = {}
    for i, c in enumerate(cat_ids.tolist()):
        by_cat.setdefault(c, []).append(i)
    pairs = []
    singles = []
    for c, idxs in sorted(by_cat.items()):
        n = len(idxs)
        for j in range(n // 2):
            pairs.append((c, idxs[2 * j], idxs[2 * j + 1]))
        if n % 2:
            singles.append((c, idxs[-1]))
    assert len(pairs) >= N_CORES, "impossible: <8 same-cat pairs among 32 batches"
    core_pairs = pairs[:N_CORES]
    # leftovers: extra pairs flatten into singles
    for c, i, j in pairs[N_CORES:]:
        singles.append((c, i))
        singles.append((c, j))
    assert len(singles) == 2 * N_CORES
    cores = []
    for ci in range(N_CORES):
        c, i, j = core_pairs[ci]
        (cb, ib), (cc, ic) = singles[2 * ci], singles[2 * ci + 1]
        cores.append(([i, j, ib, ic], [c, cb, cc]))
    return cores


def run_sharded(x, cat_ids, W, b, trace=False, **spmd_kwargs):
    """Shard, run on 8 cores, unshard. Returns (out, BassKernelResults)."""
    x = np.ascontiguousarray(np.asarray(x), dtype=np.float32)
    cat_ids = np.asarray(cat_ids).astype(np.int64)
    W = np.ascontiguousarray(np.asarray(W), dtype=np.float32)
    b = np.ascontiguousarray(np.asarray(b), dtype=np.float32)

    nc = _get_compiled()
    cores = _pack(cat_ids)

    in_maps = []
    for idx, slot_cats in cores:
        in_maps.append(
            {
                "xt": np.ascontiguousarray(x[idx].transpose(0, 2, 1).astype(np.float16)),
                "w": np.ascontiguousarray(W[slot_cats].astype(np.float16)),
                "bias": np.ascontiguousarray(b[slot_cats]),
            }
        )

    res = run_bass_kernel_spmd(
        nc, in_maps, list(range(N_CORES)), trace=trace, **spmd_kwargs
    )

    out = np.empty((B, S, H), dtype=np.float32)
    for c, (idx, _) in enumerate(cores):
        out[idx] = res.results[c]["out"].astype(np.float32)
    return out, res


def kernel(x, cat_ids, W, b):
    out, _ = run_sharded(x, cat_ids, W, b)
    return out
